# revision 4
# baseline (speedup 1.0000x reference)
"""Trainium2 Bass kernel v4 (hybrid) for the MPS window model (nn_Hankel).

Two per-tile schemes share the device, chosen so the bottleneck engines
parallelize (per-core budgets: DVE ~260us, DMA ~265us, PE ~235us):

  v-tiles (DVE-heavy, light DMA): the baseline dataflow.  Per step:
      Q = tmp (*) xrep   [128,F]   (DVE, PSUM read)
      R = G2^T Q         2 matmuls -> PSUM [128,2,F]
      V = R (*) yrep     [128,2,F] (DVE, PSUM read)
      tmp' = RED^T V     2 matmuls -> PSUM [128,F]  (replicates over j)

  z-tiles (DMA-heavy, light DVE): the host ships z = x (x) y outer
  products (512 values/col/step, fp16).  Per step:
      H = Gz^T z         4 accumulating matmuls -> PSUM [64,F] ((i,l) space)
      M = H (*) Trep     [64,F]    (DVE -> fp16 SBUF)
      Trep' = REP^T M    1 matmul -> PSUM [64,F]   (tmp'[i] replicated)
  The H-matmuls are off the serial chain, so z-tiles pipeline deeply.

Both schemes share the two PSUM pool sites (r: [128,2,F], t: [128,F])
through common alloc helpers so 8 banks suffice.
"""

import os
import numpy as np

B, L, A_IN, O_IN, RANK = 131072, 8, 16, 32, 8
NCORES = 8
NC_N = B // NCORES          # 16384 batch per core
F = 512                     # free-dim columns per tile
NT = NC_N // F              # 32 tiles per core

# Tile scheme pattern: 'v' or 'z' per tile index (repeats/truncates to NT).
PATTERN = os.environ.get("KV_PATTERN", "vzvzvzvzvzvzvzvzvzvzvzvzvzvzvzvv")
WAYS = int(os.environ.get("KV_WAYS", "2"))
PRBUFS = int(os.environ.get("KV_PRBUFS", "2"))
PTBUFS = int(os.environ.get("KV_PTBUFS", "2"))
IOBUFS = int(os.environ.get("KV_IOBUFS", "3"))
ZIOBUFS = int(os.environ.get("KV_ZIOBUFS", "4"))
WORKBUFS = int(os.environ.get("KV_WORKBUFS", "4"))
NT_OVERRIDE = os.environ.get("KV_NT")

_PROGRAM_CACHE = {}


def _scheme(it):
    return PATTERN[it % len(PATTERN)]


def _patch_wait_splitting():
    """This container's walrus permits only one sync-wait per instruction.
    Split extra waits onto inserted single-wait EventSemaphore instructions."""
    import json as _json
    import concourse.bass as b
    if getattr(b.Bass, "_wait_split_patched", False):
        return
    orig = b.Bass.to_json_bytes

    def to_json_bytes(self):
        m = _json.loads(orig(self))
        ctr = 0
        for fn in m.get("functions", []):
            for bb in fn.get("blocks", []):
                insts = bb.get("instructions")
                if not insts:
                    continue
                out = []
                for ins in insts:
                    si = ins.get("sync_info") or {}
                    waits = si.get("on_wait") or []
                    if len(waits) > 1:
                        for w in waits[:-1]:
                            ctr += 1
                            out.append({
                                "debug": ins.get("debug", 0),
                                "engine": ins["engine"],
                                "ins": [],
                                "name": f"EVWSPLIT-{ctr}",
                                "opcode": "EventSemaphore",
                                "outs": [],
                                "sync_info": {"on_update": [], "on_wait": [w]},
                            })
                        si["on_wait"] = [waits[-1]]
                    out.append(ins)
                bb["instructions"] = out
        return _json.dumps(m).encode()

    b.Bass.to_json_bytes = to_json_bytes
    b.Bass._wait_split_patched = True


def _tile_cols():
    """Column ranges (in the core's batch) for v-tiles and z-tiles, and the
    packed column offset of each tile within its scheme's dram tensors."""
    nt = int(NT_OVERRIDE) if NT_OVERRIDE else NT
    vmap, zmap = {}, {}
    nv = nz = 0
    for it in range(nt):
        if _scheme(it) == "v":
            vmap[it] = nv
            nv += 1
        else:
            zmap[it] = nz
            nz += 1
    return nt, vmap, zmap, nv, nz


def _build_program():
    import concourse.bass as bass
    import concourse.tile as tile
    from concourse import mybir
    from contextlib import ExitStack

    _patch_wait_splitting()

    fp16 = mybir.dt.float16
    fp32 = mybir.dt.float32

    nt, vmap, zmap, nv, nz = _tile_cols()
    NV, NZ = max(nv, 1) * F, max(nz, 1) * F

    nc = bass.Bass()
    xrep_d = nc.dram_tensor("xrep", [128, 7, NV], fp16, kind="ExternalInput")
    yrep_d = nc.dram_tensor("yrep", [128, 7, NV], fp16, kind="ExternalInput")
    x0_d = nc.dram_tensor("x0", [16, NV], fp16, kind="ExternalInput")
    y7_d = nc.dram_tensor("y7", [32, NV], fp16, kind="ExternalInput")
    zin_d = nc.dram_tensor("zin", [128, 4, 8, NZ], fp16, kind="ExternalInput")
    w0_d = nc.dram_tensor("w0", [16, 256], fp16, kind="ExternalInput")
    wmid_d = nc.dram_tensor("wmid", [128, 6, 256], fp16, kind="ExternalInput")
    w7_d = nc.dram_tensor("w7", [128, 32], fp16, kind="ExternalInput")
    red_d = nc.dram_tensor("red", [128, 2, 128], fp16, kind="ExternalInput")
    ones_d = nc.dram_tensor("ones32", [32, 1], fp16, kind="ExternalInput")
    wz_d = nc.dram_tensor("wz", [128, 4, 8, 64], fp16, kind="ExternalInput")
    repw_d = nc.dram_tensor("repw", [64, 64], fp16, kind="ExternalInput")
    ones64_d = nc.dram_tensor("ones64", [64, 1], fp16, kind="ExternalInput")
    out_d = nc.dram_tensor("out", [1, NC_N], fp32, kind="ExternalOutput")

    with tile.TileContext(nc) as tc, ExitStack() as ctx:
        consts = ctx.enter_context(tc.tile_pool(name="consts", bufs=1))
        io = ctx.enter_context(tc.tile_pool(name="io", bufs=IOBUFS))
        zio = ctx.enter_context(tc.tile_pool(name="zio", bufs=ZIOBUFS))
        work = ctx.enter_context(tc.tile_pool(name="work", bufs=WORKBUFS))
        pr = ctx.enter_context(tc.tile_pool(name="pr", bufs=PRBUFS, space="PSUM"))
        ptmp = ctx.enter_context(tc.tile_pool(name="ptmp", bufs=PTBUFS, space="PSUM"))
        pzh = ctx.enter_context(tc.tile_pool(name="pzh", bufs=int(os.environ.get("KV_ZHBUFS", "1")), space="PSUM"))

        def alloc_r():
            return pr.tile([128, 2, F], fp32, name="rbuf")

        def alloc_t():
            return ptmp.tile([128, F], fp32, name="tbuf")

        w0_t = consts.tile([16, 256], fp16)
        nc.gpsimd.dma_start(w0_t, w0_d[:, :])
        wmid_t = consts.tile([128, 6, 256], fp16)
        nc.gpsimd.dma_start(wmid_t, wmid_d[:, :, :])
        w7_t = consts.tile([128, 32], fp16)
        nc.gpsimd.dma_start(w7_t, w7_d[:, :])
        red_t = consts.tile([128, 2, 128], fp16)
        nc.gpsimd.dma_start(red_t, red_d[:, :, :])
        ones_t = consts.tile([32, 1], fp16)
        nc.gpsimd.dma_start(ones_t, ones_d[:, :])
        wz_t = consts.tile([128, 4, 8, 64], fp16)
        nc.gpsimd.dma_start(wz_t, wz_d[:, :, :, :])
        repw_t = consts.tile([64, 64], fp16)
        nc.gpsimd.dma_start(repw_t, repw_d[:, :])
        ones64_t = consts.tile([64, 1], fp16)
        nc.gpsimd.dma_start(ones64_t, ones64_d[:, :])

        class S:
            pass

        # ---------------- v-scheme ----------------
        def v_load(s):
            pc = slice(s.pcol * F, (s.pcol + 1) * F)
            s.xr = io.tile([128, 7, F], fp16)
            nc.sync.dma_start(s.xr, xrep_d[:, :, pc])
            s.yr = io.tile([128, 7, F], fp16)
            nc.sync.dma_start(s.yr, yrep_d[:, :, pc])
            s.x0t = io.tile([16, F], fp16)
            nc.sync.dma_start(s.x0t, x0_d[:, pc])
            s.y7t = io.tile([32, F], fp16)
            nc.sync.dma_start(s.y7t, y7_d[:, pc])

        def v_step(s, t):
            if t == 0:
                s.r = alloc_r()
                nc.tensor.matmul(s.r[:, 0, :], w0_t[:, 0:128], s.x0t, start=True, stop=True)
                nc.tensor.matmul(s.r[:, 1, :], w0_t[:, 128:256], s.x0t, start=True, stop=True)
            elif t == 7:
                s.q = work.tile([128, F], fp16, name="qbuf")
                nc.vector.tensor_mul(s.q, s.tmp, s.xr[:, 6, :])
                s.r = alloc_r()
                nc.tensor.matmul(s.r[0:32, 0, :], w7_t, s.q, start=True, stop=True)
                s.v7 = work.tile([32, F], fp16, name="v7buf")
                nc.vector.tensor_mul(s.v7, s.r[0:32, 0, :], s.y7t)
                s.tmp = alloc_t()
                nc.tensor.matmul(s.tmp[0:1, :], ones_t, s.v7, start=True, stop=True)
                return
            else:
                s.q = work.tile([128, F], fp16, name="qbuf")
                nc.vector.tensor_mul(s.q, s.tmp, s.xr[:, t - 1, :])
                s.r = alloc_r()
                nc.tensor.matmul(s.r[:, 0, :], wmid_t[:, t - 1, 0:128], s.q, start=True, stop=True)
                nc.tensor.matmul(s.r[:, 1, :], wmid_t[:, t - 1, 128:256], s.q, start=True, stop=True)
            yrb = s.yr[:, t, :].unsqueeze(1).broadcast_to([128, 2, F])
            s.v = work.tile([128, 2, F], fp16, name="vbuf")
            nc.vector.tensor_mul(s.v, s.r, yrb)
            s.tmp = alloc_t()
            nc.tensor.matmul(s.tmp, red_t[:, 0, :], s.v[:, 0, :], start=True, stop=False)
            nc.tensor.matmul(s.tmp, red_t[:, 1, :], s.v[:, 1, :], start=False, stop=True)

        # ---------------- z-scheme ----------------
        def z_load_step(s, t):
            pc = slice(s.pcol * F, (s.pcol + 1) * F)
            zt = zio.tile([128, 4, F], fp16, name="zbuf")
            nc.sync.dma_start(zt, zin_d[:, :, t, pc])
            s.zt[t] = zt

        def z_step(s, t):
            zt = s.zt[t]
            if t == 0:
                # H0 IS Trep_1 (weights already replicate tmp1[i] over l);
                # write it straight into a ptmp slot so it survives into the
                # next step without holding a pr slot (which would deadlock
                # the pool rotation).
                s.tmp = alloc_t()
                tps = s.tmp[0:64, :]
                for sub in range(4):
                    nc.tensor.matmul(tps, wz_t[:, sub, 0, :], zt[:, sub, :],
                                     start=(sub == 0), stop=(sub == 3))
                s.trep_sb = work.tile([64, F], fp16, name="trepbuf")
                nc.scalar.copy(s.trep_sb, tps)
                return
            s.h = pzh.tile([64, F], fp32, name="zhbuf")
            h = s.h
            for sub in range(4):
                nc.tensor.matmul(h, wz_t[:, sub, t, :], zt[:, sub, :],
                                 start=(sub == 0), stop=(sub == 3))
            m = work.tile([64, F], fp16, name="mbuf")
            nc.vector.tensor_mul(m, h, s.trep_sb)
            s.tmp = alloc_t()
            if t == 7:
                nc.tensor.matmul(s.tmp[0:1, :], ones64_t, m, start=True, stop=True)
            else:
                nc.tensor.matmul(s.tmp[0:64, :], repw_t, m, start=True, stop=True)
                s.trep_sb = work.tile([64, F], fp16, name="trepbuf")
                nc.scalar.copy(s.trep_sb, s.tmp[0:64, :])

        # ---------------- common tail ----------------
        def emit_out(s):
            stage = work.tile([1, F], fp32, name="stagebuf")
            nc.scalar.copy(stage, s.tmp[0:1, :])
            nc.sync.dma_start(out_d[:, s.cs], stage)

        tiles = []
        for it in range(nt):
            s = S()
            s.it = it
            s.cs = slice(it * F, (it + 1) * F)
            s.kind = _scheme(it)
            s.pcol = vmap[it] if s.kind == "v" else zmap[it]
            s.zt = {}
            tiles.append(s)

        for w0i in range(0, nt, WAYS):
            grp = tiles[w0i:w0i + WAYS]
            for s in grp:
                if s.kind == "v":
                    v_load(s)
                else:
                    for t in range(8):
                        z_load_step(s, t)
            for t in range(8):
                for s in grp:
                    if s.kind == "v":
                        v_step(s, t)
                    else:
                        z_step(s, t)
            for s in grp:
                emit_out(s)
    return nc


def _host_reference(actions, obss, Wa, ba, Wo, bo, mps0, mps_mid, mps_last):
    b, length, _ = actions.shape
    act = (actions.reshape(b * length, -1) @ Wa.T + ba).reshape(b, length, -1)
    obs = (obss.reshape(b * length, -1) @ Wo.T + bo).reshape(b, length, -1)
    tmp = np.einsum("jkl,nj,nk->nl", mps0[0], act[:, 0], obs[:, 0])
    for i in range(1, length - 1):
        tmp = np.einsum("ni,ijkl,nj,nk->nl", tmp, mps_mid[i - 1], act[:, i], obs[:, i])
    tmp = np.einsum("ni,ijkl,nj,nk->nl", tmp, mps_last, act[:, length - 1], obs[:, length - 1])
    return tmp.squeeze(-1).astype(np.float32)


def kernel(actions, obss, Wa, ba, Wo, bo, mps0, mps_mid, mps_last):
    actions = np.asarray(actions, dtype=np.float32)
    obss = np.asarray(obss, dtype=np.float32)
    Wa = np.asarray(Wa, dtype=np.float32)
    Wo = np.asarray(Wo, dtype=np.float32)
    ba = np.asarray(ba, dtype=np.float32)
    bo = np.asarray(bo, dtype=np.float32)
    if np.any(ba != 0) or np.any(bo != 0):
        return _host_reference(actions, obss, Wa, ba, Wo, bo,
                               np.asarray(mps0), np.asarray(mps_mid), np.asarray(mps_last))

    from concourse.bass_utils import run_bass_kernel_spmd

    mps0 = np.asarray(mps0, dtype=np.float32)
    mps_mid = np.asarray(mps_mid, dtype=np.float32)
    mps_last = np.asarray(mps_last, dtype=np.float32)

    G0 = np.einsum("abl,aj,bk->jkl", mps0[0], Wa, Wo)          # [16,32,8]
    Gm = np.einsum("miabl,aj,bk->mijkl", mps_mid, Wa, Wo)      # [6,8,16,32,8]
    G7 = np.einsum("iabl,aj,bk->ijkl", mps_last, Wa, Wo)       # [8,16,32,1]

    # v-scheme weights (row 16i+j, col 32l+k in l-major 128-chunks)
    w0 = np.ascontiguousarray(G0.transpose(0, 2, 1).reshape(16, 256)).astype(np.float16)
    wmid = np.ascontiguousarray(Gm.transpose(1, 2, 0, 4, 3).reshape(128, 6, 256)).astype(np.float16)
    w7 = np.ascontiguousarray(G7[:, :, :, 0].reshape(128, 32)).astype(np.float16)
    red = np.zeros((128, 2, 128), dtype=np.float16)
    for c in range(2):
        for a in range(4):
            for k in range(32):
                ip = 4 * c + a
                red[32 * a + k, c, 16 * ip:16 * ip + 16] = 1.0
    ones32 = np.ones((32, 1), dtype=np.float16)

    # z-scheme weights: wz[(jk) as (sub,128), sub, t, (i,l) = 8i+l]
    # step 0: Trep1[(i,l)] = tmp1[i] -> W[(jk),(i,l)] = G0[(jk), i]
    # steps 1..6: H[(i,l)] = Gm[t][i,:,:,l] contracted with z
    # step 7: H7 rows (i, l=0) only = G7[(jk), i]
    wz = np.zeros((512, 8, 64), dtype=np.float32)
    g0f = G0.reshape(512, 8)                                   # [(jk), l] -> tmp1[l]
    for i in range(8):
        for l in range(8):
            wz[:, 0, 8 * i + l] = g0f[:, i]                    # Trep1[(i,l)] = tmp1[i]
    for t in range(1, 7):
        gm = Gm[t - 1]                                         # [i,j,k,l]
        wz[:, t, :] = gm.transpose(1, 2, 0, 3).reshape(512, 64)  # [(jk), (i,l)]
    g7f = G7[:, :, :, 0].transpose(1, 2, 0).reshape(512, 8)    # [(jk), i]
    for i in range(8):
        wz[:, 7, 8 * i + 0] = g7f[:, i]
    wz_in = np.ascontiguousarray(
        wz.reshape(4, 128, 8, 64).transpose(1, 0, 2, 3)).astype(np.float16)  # [128,4,8,64]

    repw = np.zeros((64, 64), dtype=np.float16)
    # Trep'[(i',l')] = tmp'[i'] = sum_i M[(i, l=i')]  -> W[(i,l),(i',l')] = 1 iff l == i'
    for i in range(8):
        for l in range(8):
            for lp in range(8):
                repw[8 * i + l, 8 * l + lp] = 1.0
    ones64 = np.ones((64, 1), dtype=np.float16)

    nt, vmap, zmap, nv, nz = _tile_cols()
    NV, NZ = max(nv, 1) * F, max(nz, 1) * F

    in_maps = []
    for core in range(NCORES):
        nsl = slice(core * NC_N, (core + 1) * NC_N)
        xT = np.ascontiguousarray(actions[nsl].transpose(2, 1, 0)).astype(np.float16)  # [16,8,N]
        yT = np.ascontiguousarray(obss[nsl].transpose(2, 1, 0)).astype(np.float16)     # [32,8,N]

        vcols = np.concatenate([np.arange(it * F, (it + 1) * F)
                                for it in sorted(vmap, key=lambda i: vmap[i])]) \
            if vmap else np.arange(F)
        zcols = np.concatenate([np.arange(it * F, (it + 1) * F)
                                for it in sorted(zmap, key=lambda i: zmap[i])]) \
            if zmap else np.arange(F)

        xv = xT[:, :, vcols]
        yv = yT[:, :, vcols]
        xrep = np.ascontiguousarray(
            np.broadcast_to(xv[None, :, 1:8, :], (8, 16, 7, len(vcols))).reshape(128, 7, -1))
        yrep = np.ascontiguousarray(
            np.broadcast_to(yv[None, :, 0:7, :], (4, 32, 7, len(vcols))).reshape(128, 7, -1))

        # z build for z-columns: z[(jk), t, n] = x[j,t,n]*y[k,t,n], laid out
        # as [128, sub, t, n] with (jk) = 128*sub + row
        xz = xT[:, :, zcols].astype(np.float32)       # [16, 8, NZ]
        yz = yT[:, :, zcols].astype(np.float32)       # [32, 8, NZ]
        z = (xz[:, None, :, :] * yz[None, :, :, :])   # [16, 32, 8, NZ]
        zin = np.ascontiguousarray(
            z.reshape(4, 128, 8, len(zcols)).transpose(1, 0, 2, 3)).astype(np.float16)

        in_maps.append({
            "xrep": xrep, "yrep": yrep,
            "x0": np.ascontiguousarray(xv[:, 0, :]),
            "y7": np.ascontiguousarray(yv[:, 7, :]),
            "zin": zin,
            "w0": w0, "wmid": wmid, "w7": w7, "red": red, "ones32": ones32,
            "wz": wz_in, "repw": repw, "ones64": ones64,
        })

    if "prog" not in _PROGRAM_CACHE:
        _PROGRAM_CACHE["prog"] = _build_program()
    nc = _PROGRAM_CACHE["prog"]

    trace = bool(int(os.environ.get("KERNEL_TRACE", "0")))
    res = run_bass_kernel_spmd(nc, in_maps, core_ids=list(range(NCORES)), trace=trace)
    if trace:
        _PROGRAM_CACHE["exec_time_ns"] = res.exec_time_ns
        _PROGRAM_CACHE["trace"] = res.instructions_and_trace
    out = np.concatenate([res.results[c]["out"].reshape(-1) for c in range(NCORES)])
    return out.astype(np.float32)


if __name__ == "__main__":
    nc = _build_program()
    from concourse.timeline_sim import TimelineSim
    print("predicted ns:", TimelineSim(nc).simulate())
